# revision 12
# baseline (speedup 1.0000x reference)
"""Multi-head causal attention (N=4, T=2048, DM=1024, H=16, D=64) on 8 trn2 cores.

Sharding: core = (batch, query-half). Each core owns 8 of the 16 query blocks
of one batch, chosen as pairs {2j, B-1-2j} / {2j+1, B-2-2j} so both halves do
equal causal work. K/V for the full batch are computed on both of its cores
(no collectives anywhere; the output rows are disjoint and host-gathered).

On-chip layout: activations arrive host-transposed as x^T [dm, tok] in bf16,
so Q/K projections emit Qt/Kt [feat, tok] directly (weights are the stationary
operand) and V projects naturally [tok, feat]. Scores are computed transposed,
St[k, q] = Kt_h^T-slice . Qt_h, so the softmax reduction (over k) can ride the
wV matmul: V is stored as [V_even | ones64 | V_odd] per head pair, making the
wV matmul emit 64 replicated rows of Z = sum_k exp alongside the attention
numerator; normalization is then reciprocal + elementwise multiply (the DVE
rejects partition-broadcast APs, which is why Z is replicated via the PE).
Softmax skips max-subtraction: scaled scores are ~N(0, 0.41), exp is safe.
Causal/padding masks are per-core *data* so one SPMD program serves all cores:
additive mask tiles for the (at most one) diagonal-window block per k-block,
and a per-partition key-padding bias fused into the exp activation.
"""

import numpy as np
import ml_dtypes

import concourse.bass as bass
import concourse.mybir as mybir
from concourse import bacc
from concourse.tile import TileContext
from concourse.bass_utils import run_bass_kernel_spmd

bf16 = mybir.dt.bfloat16
f32 = mybir.dt.float32

_NC_CACHE = {}


def _query_blocks(n_blocks, half):
    """Split blocks 0..B-1 into two equal-causal-work halves."""
    a, b = [], []
    for j in range(n_blocks // 4):
        a += [2 * j, n_blocks - 1 - 2 * j]
        b += [2 * j + 1, n_blocks - 2 - 2 * j]
    return sorted(a if half == 0 else b)


def _chunks512(start, end):
    """Split [start, end) at 512-aligned boundaries (PSUM bank limit)."""
    out = []
    pos = start
    while pos < end:
        nxt = min((pos // 512 + 1) * 512, end)
        out.append((pos, nxt))
        pos = nxt
    return out


def build_kernel(tok_q, tok_k, dm, n_heads, bias_mode):
    """One SPMD core program. tok_q per-core query tokens, tok_k keys."""
    d = 64
    S = tok_q // 128          # query slots (blocks) per core
    KB = tok_k // 128         # key blocks
    assert KB == 2 * S
    F = (n_heads * d) // 128  # feature p-tiles (= head pairs HP)
    C = dm // 128             # dm contraction chunks
    HP = n_heads // 2
    assert F == HP
    FW = F * 128              # feature width (= n_heads*d)
    HPW = HP * 192            # V_aug width per key block
    gq = min(512, tok_q)
    gk = min(512, tok_k)
    NGQ = tok_q // gq
    NGK = tok_k // gk
    NFH = max(1, FW // 512)   # feature halves for V-proj psum
    fhw = FW // NFH

    nc = bacc.Bacc("TRN2", target_bir_lowering=False, debug=False)

    dmr = dm + 1 if bias_mode else dm
    xqT = nc.dram_tensor("xqT", [dmr, tok_q], bf16, kind="ExternalInput")
    xkT = nc.dram_tensor("xkT", [dmr, tok_k], bf16, kind="ExternalInput")
    xvT = nc.dram_tensor("xvT", [dmr, tok_k], bf16, kind="ExternalInput")
    Wq = nc.dram_tensor("Wq", [dmr, FW], bf16, kind="ExternalInput")
    Wk = nc.dram_tensor("Wk", [dmr, FW], bf16, kind="ExternalInput")
    Wv = nc.dram_tensor("Wv", [dmr, FW], bf16, kind="ExternalInput")
    Wo = nc.dram_tensor("Wo", [FW, dm], bf16, kind="ExternalInput")
    MASK = nc.dram_tensor("MASK", [KB, 128, 128], bf16, kind="ExternalInput")
    PAD = nc.dram_tensor("PAD", [128, KB], f32, kind="ExternalInput")
    if bias_mode:
        BO = nc.dram_tensor("BO", [1, dm], bf16, kind="ExternalInput")
    Y = nc.dram_tensor("Y", [tok_q, dm], f32, kind="ExternalOutput")
    XVT_R = xvT.ap()[0:dm, :].rearrange("(c p) t -> p c t", p=128)

    with TileContext(nc) as tc:
        # resident tiles (hold the free-closures so the pools stay alive)
        _frees = []

        def _res_tile(shape, dtype, name):
            t, fr = tc.tile(shape, dtype, name=name)
            _frees.append(fr)
            return t

        qt_all = _res_tile([128, F * tok_q], bf16, "qt_all")
        kt_all = _res_tile([128, F * tok_k], bf16, "kt_all")
        vaug = _res_tile([128, KB * HPW], bf16, "vaug")
        attn_sb = _res_tile([128, HP * tok_q], bf16, "attn_sb")
        wv_sb = _res_tile([128, C * FW], bf16, "wv_sb")
        wo_sb = _res_tile([128, F * dm], bf16, "wo_sb")
        mask_sb = _res_tile([128, KB * 128], bf16, "mask_sb")
        pad_sb = _res_tile([128, KB], f32, "pad_sb")
        vaug_r = vaug.rearrange("p (k h s) -> p k h s", k=KB, h=HP, s=192)

        for cc in range(C):
            nc.sync.dma_start(wv_sb[:, cc * FW:(cc + 1) * FW],
                              Wv[cc * 128:(cc + 1) * 128, :])
        for hp in range(F):
            nc.sync.dma_start(wo_sb[:, hp * dm:(hp + 1) * dm],
                              Wo[hp * 128:(hp + 1) * 128, :])
        nc.sync.dma_start(
            mask_sb.rearrange("p (k q) -> p k q", k=KB),
            MASK.ap().rearrange("k p q -> p k q"))
        nc.sync.dma_start(pad_sb[:], PAD[:])

        if bias_mode:
            ones_sb = _res_tile([1, 512], bf16, "ones_sb")
            nc.gpsimd.memset(ones_sb[:], 1.0)
            wqb_sb = _res_tile([1, FW], bf16, "wqb_sb")
            wkb_sb = _res_tile([1, FW], bf16, "wkb_sb")
            bv_sb = _res_tile([1, FW], bf16, "bv_sb")
            bo_sb = _res_tile([1, dm], bf16, "bo_sb")
            nc.sync.dma_start(wqb_sb[:], Wq[dm:dm + 1, :])
            nc.sync.dma_start(wkb_sb[:], Wk[dm:dm + 1, :])
            nc.sync.dma_start(bv_sb[:], Wv[dm:dm + 1, :])
            nc.sync.dma_start(bo_sb[:], BO[:])
            onesq = _res_tile([1, gq], bf16, "onesq")
            nc.gpsimd.memset(onesq[:], 1.0)
            onesk = _res_tile([1, gk], bf16, "onesk")
            nc.gpsimd.memset(onesk[:], 1.0)

        # ones columns of V_aug
        for kb in range(KB):
            nc.gpsimd.memset(vaug_r[:, kb, :, 64:128], 1.0)

        # ---- projections (Q, K transposed-out; V natural) ----
        with (
            tc.tile_pool(name="projps", space="PSUM", bufs=1) as pps,
            tc.tile_pool(name="projsb", bufs=1) as psb,
        ):
            def qk_proj(xT, W, wbias, ones_g, dst, tok, g_sz, n_g, xtag, wtag):
                for g in range(n_g):
                    accs = [pps.tile([128, g_sz], f32, name=f"acc{f}",
                                     tag=f"acc{f}", bufs=1) for f in range(F)]
                    for cc in range(C):
                        xt = psb.tile([128, g_sz], bf16, name=xtag, tag=xtag,
                                      bufs=3)
                        nc.sync.dma_start(
                            xt[:], xT[cc * 128:(cc + 1) * 128,
                                      g * g_sz:(g + 1) * g_sz])
                        wt_ = psb.tile([128, FW], bf16, name=wtag, tag=wtag,
                                       bufs=3)
                        nc.sync.dma_start(wt_[:], W[cc * 128:(cc + 1) * 128, :])
                        for f in range(F):
                            nc.tensor.matmul(
                                accs[f][:], wt_[:, f * 128:(f + 1) * 128],
                                xt[:], start=(cc == 0),
                                stop=(cc == C - 1 and not bias_mode))
                    for f in range(F):
                        if bias_mode:
                            nc.tensor.matmul(
                                accs[f][:], wbias[0:1, f * 128:(f + 1) * 128],
                                ones_g[:], start=False, stop=True)
                        nc.vector.tensor_copy(
                            dst[:, f * tok + g * g_sz:f * tok + (g + 1) * g_sz],
                            accs[f][:])

            qk_proj(xqT, Wq, wqb_sb if bias_mode else None,
                    onesq if bias_mode else None, qt_all, tok_q, gq, NGQ,
                    "xtq", "wtq")
            qk_proj(xkT, Wk, wkb_sb if bias_mode else None,
                    onesk if bias_mode else None, kt_all, tok_k, gk, NGK,
                    "xtk", "wtk")

            # V natural: [tok, feat]
            for g in range(NGK):
                xvt = psb.tile([128, C * gk], bf16, name="xvt", tag="xvt",
                               bufs=2)
                nc.sync.dma_start(
                    xvt.rearrange("p (c t) -> p c t", c=C),
                    XVT_R[:, :, g * gk:(g + 1) * gk])
                for ti in range(gk // 128):
                    kb = g * (gk // 128) + ti
                    for fh in range(NFH):
                        vps = pps.tile([128, fhw], f32, name="vps",
                                       tag=f"acc{fh}", bufs=1)
                        for cc in range(C):
                            nc.tensor.matmul(
                                vps[:],
                                xvt[:, cc * gk + ti * 128:cc * gk + (ti + 1) * 128],
                                wv_sb[:, cc * FW + fh * fhw:cc * FW + (fh + 1) * fhw],
                                start=(cc == 0),
                                stop=(cc == C - 1 and not bias_mode))
                        if bias_mode:
                            nc.tensor.matmul(
                                vps[:], onesk[0:1, 0:128],
                                bv_sb[0:1, fh * fhw:(fh + 1) * fhw],
                                start=False, stop=True)
                        # scatter into vaug: [V_even | ones | V_odd] per pair
                        hp_per_fh = fhw // 128
                        hp0 = fh * hp_per_fh
                        vps_r = vps.rearrange("p (h pa d) -> p h pa d",
                                              h=hp_per_fh, pa=2, d=64)
                        for par in range(2):
                            nc.vector.tensor_copy(
                                vaug_r[:, kb, hp0:hp0 + hp_per_fh,
                                       par * 128:par * 128 + 64],
                                vps_r[:, :, par, :])

        # ---- attention, per head ----
        with (
            tc.tile_pool(name="stps", space="PSUM", bufs=1) as stps,
            tc.tile_pool(name="atps", space="PSUM", bufs=1) as atps,
            tc.tile_pool(name="attnsb", bufs=1) as asb,
        ):
            for h in range(n_heads):
                f = h // 2
                row = (h % 2) * 64
                at = atps.tile([128, S * 128], f32, name="at", tag="at",
                               bufs=2)
                for kb in range(KB):
                    s = kb // 2
                    ncols = (S - s) * 128
                    st = stps.tile([128, S * 128], f32, name="st", tag="st",
                                   bufs=2)
                    # chunk in *tile* coords so each MM stays in one PSUM bank
                    for (a, b) in _chunks512(0, ncols):
                        nc.tensor.matmul(
                            st[:, a:b],
                            kt_all[row:row + 64,
                                   f * tok_k + kb * 128:f * tok_k + (kb + 1) * 128],
                            qt_all[row:row + 64,
                                   f * tok_q + s * 128 + a:f * tok_q + s * 128 + b],
                            start=True, stop=True)
                    # causal/window mask on the first (diagonal-slot) block
                    nc.vector.tensor_add(
                        st[:, 0:128], st[:, 0:128],
                        mask_sb[:, kb * 128:(kb + 1) * 128])
                    wt = asb.tile([128, S * 128], bf16, name="wt", tag="wt",
                                  bufs=3)
                    nc.scalar.activation(
                        wt[:, 0:ncols], st[:, 0:ncols],
                        mybir.ActivationFunctionType.Exp,
                        bias=pad_sb[:, kb:kb + 1], scale=0.125)
                    # wV (+Z replication): chunked, closing finished slots
                    off = kb * HPW + (h // 2) * 192 + (h % 2) * 64
                    lhs = vaug[:, off:off + 128]
                    # stop only on the last write to a PSUM bank: slot s
                    # closes at odd kb, and it is the bank's final writer
                    # iff it is the last slot mapped to that bank.
                    if kb % 2 == 1:
                        s_is_bank_last = s == min(4 * (s // 4) + 3, S - 1)
                        spans = [(s * 128, s * 128 + 128, s_is_bank_last)]
                        spans += [(a, b, False)
                                  for (a, b) in _chunks512((s + 1) * 128,
                                                           S * 128)]
                    else:
                        spans = [(a, b, False)
                                 for (a, b) in _chunks512(s * 128, S * 128)]
                    for (a, b, stop) in spans:
                        nc.tensor.matmul(
                            at[:, a:b], lhs, wt[:, a - s * 128:b - s * 128],
                            start=(kb == 0), stop=stop)
                # normalize: rows [row..row+64) = numerator for even h,
                # [64-row..) holds Z replicated; for odd h they swap.
                zrow = 64 - row
                rz = asb.tile([64, S * 128], f32, name="rz", tag="rz", bufs=2)
                nc.vector.reciprocal(rz[:], at[zrow:zrow + 64, :])
                nc.vector.tensor_mul(
                    attn_sb[row:row + 64, f * tok_q:(f + 1) * tok_q],
                    at[row:row + 64, :], rz[:])

        # ---- output projection ----
        with (
            tc.tile_pool(name="ops", space="PSUM", bufs=1) as ops,
            tc.tile_pool(name="osb", bufs=1) as osb,
        ):
            ndh = max(1, dm // 512)
            dhw = dm // ndh
            for tt in range(tok_q // 128):
                for dh in range(ndh):
                    yps = ops.tile([128, dhw], f32, name="yps", tag="yps",
                                   bufs=3)
                    for hp in range(HP):
                        nc.tensor.matmul(
                            yps[:],
                            attn_sb[:, hp * tok_q + tt * 128:hp * tok_q + (tt + 1) * 128],
                            wo_sb[:, hp * dm + dh * dhw:hp * dm + (dh + 1) * dhw],
                            start=(hp == 0),
                            stop=(hp == HP - 1 and not bias_mode))
                    if bias_mode:
                        nc.tensor.matmul(
                            yps[:], ones_sb[0:1, 0:128],
                            bo_sb[0:1, dh * dhw:(dh + 1) * dhw],
                            start=False, stop=True)
                    ysb = osb.tile([128, dhw], f32, name="ysb", tag="ysb",
                                   bufs=3)
                    nc.vector.tensor_copy(ysb[:], yps[:])
                    nc.sync.dma_start(
                        Y[tt * 128:(tt + 1) * 128, dh * dhw:(dh + 1) * dhw],
                        ysb[:])

        # release resident single-tile pools in LIFO order
        for fr in reversed(_frees):
            fr()

    nc.compile()
    return nc


def _get_nc(tok_q, tok_k, dm, n_heads, bias_mode):
    key = (tok_q, tok_k, dm, n_heads, bias_mode)
    if key not in _NC_CACHE:
        _NC_CACHE[key] = build_kernel(*key)
    return _NC_CACHE[key]


def _bf16(a):
    return np.ascontiguousarray(a.astype(ml_dtypes.bfloat16))


def make_core_inputs(xq, xk, xv, padding_mask, Wq, bq, Wk, bk, Wv, bv, Wo, bo,
                     n, half, bias_mode):
    """Host-side shard prep for core (n, half)."""
    T, dm = xk.shape[1], xk.shape[2]
    B = T // 128
    blocks = _query_blocks(B, half)
    S = len(blocks)
    KB = B
    rows = np.concatenate([np.arange(b * 128, (b + 1) * 128) for b in blocks])
    xq_c = np.asarray(xq[n])[rows]                      # [tok_q, dm]
    xqT = np.ascontiguousarray(xq_c.T)
    xkT = np.ascontiguousarray(np.asarray(xk[n]).T)     # [dm, T]
    xvT = np.ascontiguousarray(np.asarray(xv[n]).T)

    def aug_x(xT):
        if not bias_mode:
            return _bf16(xT)
        return _bf16(np.vstack([xT, np.ones((1, xT.shape[1]), np.float32)]))

    def aug_w(W, b):
        if not bias_mode:
            return _bf16(W)
        return _bf16(np.vstack([W, np.asarray(b)[None, :]]))

    # causal masks: for key-block kb, the (single) slot kb//2 may need masking
    masks = np.zeros((KB, 128, 128), np.float32)
    ar = np.arange(128)
    for kb in range(KB):
        qb = blocks[kb // 2]
        if kb == qb:
            masks[kb] = np.where(ar[:, None] > ar[None, :], -1e9, 0.0)
        elif kb > qb:
            masks[kb] = -1e9
    pad = np.where(np.asarray(padding_mask[n]) == 0, -1e9, 0.0).astype(
        np.float32).reshape(KB, 128).T.copy()            # [128, KB]

    ins = {
        "xqT": aug_x(xqT), "xkT": aug_x(xkT), "xvT": aug_x(xvT),
        "Wq": aug_w(Wq, bq), "Wk": aug_w(Wk, bk), "Wv": aug_w(Wv, bv),
        "Wo": _bf16(np.asarray(Wo)),
        "MASK": masks.astype(ml_dtypes.bfloat16),
        "PAD": np.ascontiguousarray(pad),
    }
    if bias_mode:
        ins["BO"] = _bf16(np.asarray(bo)[None, :])
    return ins, blocks


def kernel(**inputs):
    xq = np.asarray(inputs["xq"], np.float32)
    xk = np.asarray(inputs["xk"], np.float32)
    xv = np.asarray(inputs["xv"], np.float32)
    pm = np.asarray(inputs["padding_mask"])
    Wq, bq = np.asarray(inputs["Wq"], np.float32), np.asarray(inputs["bq"], np.float32)
    Wk, bk = np.asarray(inputs["Wk"], np.float32), np.asarray(inputs["bk"], np.float32)
    Wv, bv = np.asarray(inputs["Wv"], np.float32), np.asarray(inputs["bv"], np.float32)
    Wo, bo = np.asarray(inputs["Wo"], np.float32), np.asarray(inputs["bo"], np.float32)

    N, T, dm = xq.shape
    n_heads = Wq.shape[1] // 64
    bias_mode = any(float(np.abs(b).max()) > 0 for b in (bq, bk, bv, bo))

    n_cores = 8
    assert N * 2 == n_cores
    nc = _get_nc(T // 2, T, dm, n_heads, bias_mode)

    in_maps, block_list = [], []
    for c in range(n_cores):
        ins, blocks = make_core_inputs(
            xq, xk, xv, pm, Wq, bq, Wk, bk, Wv, bv, Wo, bo,
            c // 2, c % 2, bias_mode)
        in_maps.append(ins)
        block_list.append(blocks)

    res = run_bass_kernel_spmd(nc, in_maps, list(range(n_cores)))

    out = np.empty((N, T, dm), np.float32)
    for c in range(n_cores):
        y = res.results[c]["Y"]
        for i, b in enumerate(block_list[c]):
            out[c // 2, b * 128:(b + 1) * 128, :] = y[i * 128:(i + 1) * 128, :]
    return out


# revision 26
# speedup vs baseline: 1.2559x; 1.2559x over previous
"""Multi-head causal attention (N=4, T=2048, DM=1024, H=16, D=64) on 8 trn2 cores.

Sharding: core = (batch, query-half). Each core owns 8 of the 16 query blocks
of one batch, chosen as pairs {2j, B-1-2j} / {2j+1, B-2-2j} so both halves do
equal causal work. K/V for the full batch are computed on both of its cores
(no collectives anywhere; the output rows are disjoint and host-gathered).

On-chip layout: activations arrive host-transposed as x^T [dm, tok] in bf16,
so Q/K projections emit Qt/Kt [feat, tok] directly (weights are the stationary
operand) and V projects naturally [tok, feat]. Scores are computed transposed,
St[k, q] = Kt_h^T-slice . Qt_h, so the softmax reduction (over k) can ride the
wV matmul: V is stored as [V_even | ones64 | V_odd] per head pair, making the
wV matmul emit 64 replicated rows of Z = sum_k exp alongside the attention
numerator; normalization is then reciprocal + elementwise multiply (the DVE
rejects partition-broadcast APs, which is why Z is replicated via the PE).
Softmax skips max-subtraction: scaled scores are ~N(0, 0.41), exp is safe.
Causal/padding masks are per-core *data* so one SPMD program serves all cores:
additive mask tiles for the (at most one) diagonal-window block per k-block,
and a per-partition key-padding bias fused into the exp activation.
"""

import numpy as np
import ml_dtypes

import concourse.bass as bass
import concourse.mybir as mybir
from concourse import bacc
from concourse.tile import TileContext
from concourse.bass_utils import run_bass_kernel_spmd

bf16 = mybir.dt.bfloat16
f32 = mybir.dt.float32

_NC_CACHE = {}


def _query_blocks(n_blocks, half):
    """Split blocks 0..B-1 into two equal-causal-work halves."""
    a, b = [], []
    for j in range(n_blocks // 4):
        a += [2 * j, n_blocks - 1 - 2 * j]
        b += [2 * j + 1, n_blocks - 2 - 2 * j]
    return sorted(a if half == 0 else b)


def _chunks512(start, end):
    """Split [start, end) at 512-aligned boundaries (PSUM bank limit)."""
    out = []
    pos = start
    while pos < end:
        nxt = min((pos // 512 + 1) * 512, end)
        out.append((pos, nxt))
        pos = nxt
    return out


def build_kernel(tok_q, tok_k, dm, n_heads, bias_mode):
    """One SPMD core program. tok_q per-core query tokens, tok_k keys."""
    d = 64
    S = tok_q // 128          # query slots (blocks) per core
    KB = tok_k // 128         # key blocks
    assert KB == 2 * S
    F = (n_heads * d) // 128  # feature p-tiles (= head pairs HP)
    C = dm // 128             # dm contraction chunks
    HP = n_heads // 2
    assert F == HP
    FW = F * 128              # feature width (= n_heads*d)
    HPW = HP * 192            # V_aug width per key block
    gq = min(512, tok_q)
    gk = min(512, tok_k)
    NGQ = tok_q // gq
    NGK = tok_k // gk
    NFH = max(1, FW // 512)   # feature halves for V-proj psum
    fhw = FW // NFH

    nc = bacc.Bacc("TRN2", target_bir_lowering=False, debug=False)

    dmr = dm + 1 if bias_mode else dm
    xqT = nc.dram_tensor("xqT", [dmr, tok_q], bf16, kind="ExternalInput")
    xkT = nc.dram_tensor("xkT", [dmr, tok_k], bf16, kind="ExternalInput")
    xvT = nc.dram_tensor("xvT", [dmr, tok_k], bf16, kind="ExternalInput")
    Wq = nc.dram_tensor("Wq", [dmr, FW], bf16, kind="ExternalInput")
    Wk = nc.dram_tensor("Wk", [dmr, FW], bf16, kind="ExternalInput")
    Wv = nc.dram_tensor("Wv", [dmr, FW], bf16, kind="ExternalInput")
    Wo = nc.dram_tensor("Wo", [FW, dm], bf16, kind="ExternalInput")
    MASK = nc.dram_tensor("MASK", [KB, 128, 128], bf16, kind="ExternalInput")
    PAD = nc.dram_tensor("PAD", [128, KB], f32, kind="ExternalInput")
    if bias_mode:
        BO = nc.dram_tensor("BO", [1, dm], bf16, kind="ExternalInput")
    Y = nc.dram_tensor("Y", [tok_q, dm], f32, kind="ExternalOutput")
    XVT_R = xvT.ap()[0:dm, :].rearrange("(c p) t -> p c t", p=128)

    with TileContext(nc) as tc:
        # resident tiles (hold the free-closures so the pools stay alive)
        _frees = []

        def _res_tile(shape, dtype, name):
            t, fr = tc.tile(shape, dtype, name=name)
            _frees.append(fr)
            return t

        qt_all = _res_tile([128, F * tok_q], bf16, "qt_all")
        kt_all = _res_tile([128, F * tok_k], bf16, "kt_all")
        vaug = _res_tile([128, KB * HPW], bf16, "vaug")
        attn_sb = _res_tile([128, HP * tok_q], bf16, "attn_sb")
        wv_sb = _res_tile([128, C * FW], bf16, "wv_sb")
        wo_sb = _res_tile([128, F * dm], bf16, "wo_sb")
        mask_sb = _res_tile([128, KB * 128], bf16, "mask_sb")
        pad_sb = _res_tile([128, KB], f32, "pad_sb")
        vaug_r = vaug.rearrange("p (k h s) -> p k h s", k=KB, h=HP, s=192)

        if bias_mode:
            ones_sb = _res_tile([1, 512], bf16, "ones_sb")
            nc.gpsimd.memset(ones_sb[:], 1.0)
            wqb_sb = _res_tile([1, FW], bf16, "wqb_sb")
            wkb_sb = _res_tile([1, FW], bf16, "wkb_sb")
            bv_sb = _res_tile([1, FW], bf16, "bv_sb")
            bo_sb = _res_tile([1, dm], bf16, "bo_sb")
            nc.sync.dma_start(wqb_sb[:], Wq[dm:dm + 1, :])
            nc.sync.dma_start(wkb_sb[:], Wk[dm:dm + 1, :])
            nc.sync.dma_start(bv_sb[:], Wv[dm:dm + 1, :])
            nc.sync.dma_start(bo_sb[:], BO[:])
            onesq = _res_tile([1, gq], bf16, "onesq")
            nc.gpsimd.memset(onesq[:], 1.0)
            onesk = _res_tile([1, gk], bf16, "onesk")
            nc.gpsimd.memset(onesk[:], 1.0)

        # ---- projections (Q, K transposed-out; V natural) ----
        with (
            tc.tile_pool(name="projps", space="PSUM", bufs=1) as pps,
            tc.tile_pool(name="projsb", bufs=1) as psb,
        ):
            def qk_proj(xT, W, wbias, ones_g, dst, tok, g_sz, n_g, xtag, wtag):
                # two f-half passes per group: pass 1 runs while pass 0's
                # accumulators evacuate, so group-to-group PSUM reuse never
                # stalls the PE
                FH = max(1, F // 2)
                xt_r = xT.ap()[0:dm, :].rearrange("(c p) t -> p c t", p=128)
                for g in range(n_g):
                    xg = psb.tile([128, C * g_sz], bf16, name=xtag, tag=xtag,
                                  bufs=2)
                    for cc in range(C):
                        nc.sync.dma_start(
                            xg[:, cc * g_sz:(cc + 1) * g_sz],
                            xt_r[:, cc, g * g_sz:(g + 1) * g_sz])
                    for hf in range(F // FH):
                        accs = [pps.tile([128, g_sz], f32, name="acc",
                                         tag=f"acc{hf * FH + f}", bufs=1)
                                for f in range(FH)]
                        for cc in range(C):
                            wt_ = psb.tile([128, FH * 128], bf16, name=wtag,
                                           tag=wtag, bufs=6)
                            nc.sync.dma_start(
                                wt_[:],
                                W[cc * 128:(cc + 1) * 128,
                                  hf * FH * 128:(hf + 1) * FH * 128])
                            for f in range(FH):
                                nc.tensor.matmul(
                                    accs[f][:], wt_[:, f * 128:(f + 1) * 128],
                                    xg[:, cc * g_sz:(cc + 1) * g_sz],
                                    start=(cc == 0),
                                    stop=(cc == C - 1 and not bias_mode))
                        for f in range(FH):
                            fa = hf * FH + f
                            if bias_mode:
                                nc.tensor.matmul(
                                    accs[f][:],
                                    wbias[0:1, fa * 128:(fa + 1) * 128],
                                    ones_g[:], start=False, stop=True)
                            nc.vector.tensor_copy(
                                dst[:, fa * tok + g * g_sz:
                                    fa * tok + (g + 1) * g_sz],
                                accs[f][:])

            qk_proj(xqT, Wq, wqb_sb if bias_mode else None,
                    onesq if bias_mode else None, qt_all, tok_q, gq, NGQ,
                    "xg", "wtg")
            qk_proj(xkT, Wk, wkb_sb if bias_mode else None,
                    onesk if bias_mode else None, kt_all, tok_k, gk, NGK,
                    "xg", "wtg")

            # V natural: [tok, feat]
            for cc in range(C):
                nc.sync.dma_start(wv_sb[:, cc * FW:(cc + 1) * FW],
                                  Wv[cc * 128:(cc + 1) * 128, :])
            for kb in range(KB):
                nc.gpsimd.memset(vaug_r[:, kb, :, 64:128], 1.0)
            for g in range(NGK):
                xvt = psb.tile([128, C * gk], bf16, name="xvt", tag="xg",
                               bufs=2)
                nc.sync.dma_start(
                    xvt.rearrange("p (c t) -> p c t", c=C),
                    XVT_R[:, :, g * gk:(g + 1) * gk])
                for ti in range(gk // 128):
                    kb = g * (gk // 128) + ti
                    for fh in range(NFH):
                        vps = pps.tile([128, fhw], f32, name="vps",
                                       tag=f"acc{(kb * NFH + fh) % F}", bufs=1)
                        for cc in range(C):
                            nc.tensor.matmul(
                                vps[:],
                                xvt[:, cc * gk + ti * 128:cc * gk + (ti + 1) * 128],
                                wv_sb[:, cc * FW + fh * fhw:cc * FW + (fh + 1) * fhw],
                                start=(cc == 0),
                                stop=(cc == C - 1 and not bias_mode))
                        if bias_mode:
                            nc.tensor.matmul(
                                vps[:], onesk[0:1, 0:128],
                                bv_sb[0:1, fh * fhw:(fh + 1) * fhw],
                                start=False, stop=True)
                        # scatter into vaug: [V_even | ones | V_odd] per pair
                        hp_per_fh = fhw // 128
                        hp0 = fh * hp_per_fh
                        vps_r = vps.rearrange("p (h pa d) -> p h pa d",
                                              h=hp_per_fh, pa=2, d=64)
                        for par in range(2):
                            nc.vector.tensor_copy(
                                vaug_r[:, kb, hp0:hp0 + hp_per_fh,
                                       par * 128:par * 128 + 64],
                                vps_r[:, :, par, :])

        # ---- attention, per head ----
        # One exp per kb over the whole suffix (ACT per-op overhead is
        # ~200ns, so fewer/bigger ACT ops win).  The causal mask is applied
        # *multiplicatively after* exp, on only the 128 diagonal-slot
        # columns of wt (binary bf16 mask, DVE 2x mode) -- so the large
        # wV chunks never wait on the DVE.
        nc.sync.dma_start(
            mask_sb.rearrange("p (k q) -> p k q", k=KB),
            MASK.ap().rearrange("k p q -> p k q"))
        nc.sync.dma_start(pad_sb[:], PAD[:])
        with (
            tc.tile_pool(name="stps", space="PSUM", bufs=1) as stps,
            tc.tile_pool(name="atps", space="PSUM", bufs=1) as atps,
            tc.tile_pool(name="attnsb", bufs=1) as asb,
        ):
            for h in range(n_heads):
                f = h // 2
                row = (h % 2) * 64
                at = atps.tile([128, S * 128], f32, name="at", tag="at",
                               bufs=2)
                voff0 = (h // 2) * 192 + (h % 2) * 64
                for kb in range(KB):
                    s = kb // 2
                    ncols = (S - s) * 128
                    lhs_kt = kt_all[row:row + 64,
                                    f * tok_k + kb * 128:f * tok_k + (kb + 1) * 128]
                    lhs_v = vaug[:, kb * HPW + voff0:kb * HPW + voff0 + 128]
                    s_is_bank_last = s == min(4 * (s // 4) + 3, S - 1)
                    st = stps.tile([128, S * 128], f32, name="st", tag="st",
                                   bufs=2)
                    # scores: chunk in tile coords (PSUM bank limit)
                    for (a, b) in _chunks512(0, ncols):
                        nc.tensor.matmul(
                            st[:, a:b], lhs_kt,
                            qt_all[row:row + 64,
                                   f * tok_q + s * 128 + a:f * tok_q + s * 128 + b],
                            start=True, stop=True)
                    wt = asb.tile([128, S * 128], bf16, name="wt", tag="wt",
                                  bufs=4)
                    nc.scalar.activation(
                        wt[:, 0:ncols], st[:, 0:ncols],
                        mybir.ActivationFunctionType.Exp,
                        bias=pad_sb[:, kb:kb + 1], scale=0.125)
                    # binary causal mask on the diagonal-slot columns only
                    nc.vector.tensor_mul(
                        wt[:, 0:128], wt[:, 0:128],
                        mask_sb[:, kb * 128:(kb + 1) * 128])
                    # wV accumulation; stop on the bank's final writer
                    # (slot s closing at odd kb, s last slot of its bank)
                    if kb % 2 == 1 and s_is_bank_last:
                        spans = [(s * 128, s * 128 + 128, True)]
                        spans += [(a, b, False)
                                  for (a, b) in _chunks512((s + 1) * 128,
                                                           S * 128)]
                    else:
                        spans = [(a, b, False)
                                 for (a, b) in _chunks512(s * 128, S * 128)]
                    for (a, b, stop) in spans:
                        nc.tensor.matmul(
                            at[:, a:b], lhs_v, wt[:, a - s * 128:b - s * 128],
                            start=(kb == 0 and a % 512 == 0), stop=stop)
                # normalize: rows [row..row+64) = numerator for even h,
                # [64-row..) holds Z replicated; for odd h they swap.
                zrow = 64 - row
                rz = asb.tile([64, S * 128], f32, name="rz", tag="rz", bufs=2)
                nc.vector.reciprocal(rz[:], at[zrow:zrow + 64, :])
                nc.vector.tensor_mul(
                    attn_sb[row:row + 64, f * tok_q:(f + 1) * tok_q],
                    at[row:row + 64, :], rz[:])

        # ---- output projection ----
        for hp in range(F):
            nc.sync.dma_start(wo_sb[:, hp * dm:(hp + 1) * dm],
                              Wo[hp * 128:(hp + 1) * 128, :])
        with (
            tc.tile_pool(name="ops", space="PSUM", bufs=1) as ops,
            tc.tile_pool(name="osb", bufs=1) as osb,
        ):
            ndh = max(1, dm // 512)
            dhw = dm // ndh
            for tt in range(tok_q // 128):
                for dh in range(ndh):
                    yps = ops.tile([128, dhw], f32, name="yps", tag="yps",
                                   bufs=3)
                    for hp in range(HP):
                        nc.tensor.matmul(
                            yps[:],
                            attn_sb[:, hp * tok_q + tt * 128:hp * tok_q + (tt + 1) * 128],
                            wo_sb[:, hp * dm + dh * dhw:hp * dm + (dh + 1) * dhw],
                            start=(hp == 0),
                            stop=(hp == HP - 1 and not bias_mode))
                    if bias_mode:
                        nc.tensor.matmul(
                            yps[:], ones_sb[0:1, 0:128],
                            bo_sb[0:1, dh * dhw:(dh + 1) * dhw],
                            start=False, stop=True)
                    ysb = osb.tile([128, dhw], f32, name="ysb", tag="ysb",
                                   bufs=3)
                    nc.vector.tensor_copy(ysb[:], yps[:])
                    nc.sync.dma_start(
                        Y[tt * 128:(tt + 1) * 128, dh * dhw:(dh + 1) * dhw],
                        ysb[:])

        # release resident single-tile pools in LIFO order
        for fr in reversed(_frees):
            fr()

    nc.compile()
    return nc


def _get_nc(tok_q, tok_k, dm, n_heads, bias_mode):
    key = (tok_q, tok_k, dm, n_heads, bias_mode)
    if key not in _NC_CACHE:
        _NC_CACHE[key] = build_kernel(*key)
    return _NC_CACHE[key]


def _bf16(a):
    return np.ascontiguousarray(a.astype(ml_dtypes.bfloat16))


def make_core_inputs(xq, xk, xv, padding_mask, Wq, bq, Wk, bk, Wv, bv, Wo, bo,
                     n, half, bias_mode):
    """Host-side shard prep for core (n, half)."""
    T, dm = xk.shape[1], xk.shape[2]
    B = T // 128
    blocks = _query_blocks(B, half)
    S = len(blocks)
    KB = B
    rows = np.concatenate([np.arange(b * 128, (b + 1) * 128) for b in blocks])
    xq_c = np.asarray(xq[n])[rows]                      # [tok_q, dm]
    xqT = np.ascontiguousarray(xq_c.T)
    xkT = np.ascontiguousarray(np.asarray(xk[n]).T)     # [dm, T]
    xvT = np.ascontiguousarray(np.asarray(xv[n]).T)

    def aug_x(xT):
        if not bias_mode:
            return _bf16(xT)
        return _bf16(np.vstack([xT, np.ones((1, xT.shape[1]), np.float32)]))

    def aug_w(W, b):
        if not bias_mode:
            return _bf16(W)
        return _bf16(np.vstack([W, np.asarray(b)[None, :]]))

    # binary causal masks (multiplied into exp output): for key-block kb,
    # only the diagonal slot kb//2 may need masking
    masks = np.ones((KB, 128, 128), np.float32)
    ar = np.arange(128)
    for kb in range(KB):
        qb = blocks[kb // 2]
        if kb == qb:
            masks[kb] = (ar[:, None] <= ar[None, :]).astype(np.float32)
        elif kb > qb:
            masks[kb] = 0.0
    pad = np.where(np.asarray(padding_mask[n]) == 0, -1e9, 0.0).astype(
        np.float32).reshape(KB, 128).T.copy()            # [128, KB]

    ins = {
        "xqT": aug_x(xqT), "xkT": aug_x(xkT), "xvT": aug_x(xvT),
        "Wq": aug_w(Wq, bq), "Wk": aug_w(Wk, bk), "Wv": aug_w(Wv, bv),
        "Wo": _bf16(np.asarray(Wo)),
        "MASK": masks.astype(ml_dtypes.bfloat16),
        "PAD": np.ascontiguousarray(pad),
    }
    if bias_mode:
        ins["BO"] = _bf16(np.asarray(bo)[None, :])
    return ins, blocks


def kernel(**inputs):
    xq = np.asarray(inputs["xq"], np.float32)
    xk = np.asarray(inputs["xk"], np.float32)
    xv = np.asarray(inputs["xv"], np.float32)
    pm = np.asarray(inputs["padding_mask"])
    Wq, bq = np.asarray(inputs["Wq"], np.float32), np.asarray(inputs["bq"], np.float32)
    Wk, bk = np.asarray(inputs["Wk"], np.float32), np.asarray(inputs["bk"], np.float32)
    Wv, bv = np.asarray(inputs["Wv"], np.float32), np.asarray(inputs["bv"], np.float32)
    Wo, bo = np.asarray(inputs["Wo"], np.float32), np.asarray(inputs["bo"], np.float32)

    N, T, dm = xq.shape
    n_heads = Wq.shape[1] // 64
    bias_mode = any(float(np.abs(b).max()) > 0 for b in (bq, bk, bv, bo))

    n_cores = 8
    assert N * 2 == n_cores
    nc = _get_nc(T // 2, T, dm, n_heads, bias_mode)

    in_maps, block_list = [], []
    for c in range(n_cores):
        ins, blocks = make_core_inputs(
            xq, xk, xv, pm, Wq, bq, Wk, bk, Wv, bv, Wo, bo,
            c // 2, c % 2, bias_mode)
        in_maps.append(ins)
        block_list.append(blocks)

    res = run_bass_kernel_spmd(nc, in_maps, list(range(n_cores)))

    out = np.empty((N, T, dm), np.float32)
    for c in range(n_cores):
        y = res.results[c]["Y"]
        for i, b in enumerate(block_list[c]):
            out[c // 2, b * 128:(b + 1) * 128, :] = y[i * 128:(i + 1) * 128, :]
    return out
